# revision 1
# baseline (speedup 1.0000x reference)
"""Trainium2 Bass kernel for nn_CaMoE_Block (MoE routing block).

Strategy (8 NeuronCores):
  Launch 1 — data-parallel over tokens (8192 tokens / 8 cores):
    LN1 -> gated attention projections (TF32 matmuls on PE) -> residual ->
    LN2 pre-affine. Outputs x2, z2 (normalized pre-affine), state^T (bf16).
    LN affines are folded into the weight matrices on the host (z @ (w*W) +
    b@W), which keeps the device side affine-free.
  Host — routing: h = z2*w + b, Q = h @ [conf|diff|affinity] in fp32 BLAS,
    bids/argmax, borderline tokens (small top-2 gap) recomputed exactly in
    fp32 reference order; per-expert token packing with fixed per-core
    capacities (zero-padded), host computes any overflow exactly.
  Launch 2 — expert-parallel: each core gets one RWKV expert's K/V (bf16)
    plus the shared transformer expert weights; computes squared-ReLU FFN
    for up to CAP_R packed tokens and the state-gated transformer expert
    for up to CAP_T tokens.
  Host — scale by straight-through confidence and scatter-add the residual.
"""

import os
import sys

sys.path.insert(0, "/opt/trn_rl_repo")

from contextlib import ExitStack

import ml_dtypes
import numpy as np

import concourse.bacc as bacc
import concourse.tile as tile
from concourse import mybir
from concourse.bass_utils import run_bass_kernel_spmd
from concourse.masks import make_identity

F32 = mybir.dt.float32
F32R = mybir.dt.float32r
BF16 = mybir.dt.bfloat16
BF16_NP = ml_dtypes.bfloat16
AF = mybir.ActivationFunctionType

B, T, C = 4, 2048, 1024
N = B * T                      # 8192 tokens
NCORES = 8
TLOC = N // NCORES             # 1024 tokens per core
H = 4 * C                      # 4096
CAP_A = 384                    # rwkv slot-A tokens per core in launch 2
CAP_B = 256                    # rwkv slot-B tokens per core in launch 2
CAP_R = CAP_A + CAP_B          # 640 rwkv tokens per core total
CAP_T = 448                    # transformer tokens per core in launch 2
MARGIN = 3e-3                  # top-2 bid gap below which host recomputes
LN_EPS = 1e-5

# populated when BASS_MOE_TRACE=1: [launch1_ns, launch2_ns]
LAST_EXEC_NS = []

_CACHE = {}


def _trace_enabled():
    return bool(int(os.environ.get("BASS_MOE_TRACE", "0")))


def _install_trace_shims():
    """This image lacks antenv.axon_hooks; synthesize it so trace=True works."""
    import types

    import antenv
    import concourse.bass_utils as bass_utils

    if "antenv.axon_hooks" not in sys.modules:
        from trn_agent_boot.trn_boot import _ntff_profile_via_ctypes

        mod = types.ModuleType("antenv.axon_hooks")
        hook = _ntff_profile_via_ctypes("/opt/axon/libaxon_pjrt.so")
        mod.get_axon_ntff_profile_hook = lambda: hook
        mod.set_axon_ntff_profile_hook = lambda h: None
        sys.modules["antenv.axon_hooks"] = mod
        antenv.axon_hooks = mod
    bass_utils.upload_artifacts = lambda tmpdir: "local://" + tmpdir


# ---------------------------------------------------------------- launch 1


def _build_launch1(zero_b=False):
    nc = bacc.Bacc()
    x = nc.declare_dram_parameter("x", [TLOC, C], F32, isOutput=False)
    # weights pre-chunked on host: [m, p, k*128+c] with element W[k*128+p, m*128+c]
    # declared F32R: host pre-rounds to TF32, so no on-device cast is needed
    wr = nc.declare_dram_parameter("wr", [C // 128, 128, C], F32R, isOutput=False)
    wv = nc.declare_dram_parameter("wv", [C // 128, 128, C], F32R, isOutput=False)
    wo = nc.declare_dram_parameter("wo", [C // 128, 128, C], F32R, isOutput=False)
    brows = nc.declare_dram_parameter("brows", [3, C], F32, isOutput=False)
    x2 = nc.declare_dram_parameter("x2", [TLOC, C], F32, isOutput=True)
    z2 = nc.declare_dram_parameter("z2", [TLOC, C], F32, isOutput=True)
    z1t = nc.declare_dram_parameter("z1t", [C, TLOC], BF16, isOutput=True)

    NT = TLOC // 128           # 8 token tiles
    NK = C // 128              # 8 contraction chunks

    with tile.TileContext(nc) as tc, ExitStack() as ctx:
        const = ctx.enter_context(tc.tile_pool(name="const", bufs=1))
        big = ctx.enter_context(tc.tile_pool(name="big", bufs=1))
        io = ctx.enter_context(tc.tile_pool(name="io", bufs=3))
        wpool = ctx.enter_context(tc.tile_pool(name="wp", bufs=3))
        stat = ctx.enter_context(tc.tile_pool(name="stat", bufs=6))
        pmm = ctx.enter_context(tc.tile_pool(name="pmm", bufs=4, space="PSUM"))
        ptr = ctx.enter_context(tc.tile_pool(name="ptr", bufs=3, space="PSUM"))

        eps_t = const.tile([128, 1], F32)
        nc.vector.memset(eps_t, LN_EPS)
        ident = const.tile([128, 128], F32)
        make_identity(nc, ident)
        identr = const.tile([128, 128], F32R)
        nc.vector.tensor_copy(out=identr, in_=ident)
        btile = const.tile([128, 3, 8], F32)
        nc.sync.dma_start(out=btile, in_=brows.rearrange("w (m p) -> p w m", p=128))

        xfull = big.tile([128, NT, C], F32)
        xr_ap = x.rearrange("(i p) c -> p i c", p=128)
        for i in range(NT):
            nc.sync.dma_start(out=xfull[:, i, :], in_=xr_ap[:, i, :])

        zT = big.tile([128, NK, TLOC], F32R, tag="zT_attB")

        def layer_norm_pre(xt, tag):
            """-> z = (x - mean) * rstd as a fresh [128, C] f32 tile."""
            stats = stat.tile([128, 2, 6], F32, tag=f"st_{tag}")
            nc.vector.bn_stats(out=stats[:, 0, :], in_=xt[:, 0:512])
            nc.vector.bn_stats(out=stats[:, 1, :], in_=xt[:, 512:1024])
            mv = stat.tile([128, 2], F32, tag=f"mv_{tag}")
            nc.vector.bn_aggr(out=mv, in_=stats)
            rstd = stat.tile([128, 1], F32, tag=f"rs_{tag}")
            nc.scalar.activation(out=rstd, in_=mv[:, 1:2], func=AF.Sqrt, bias=eps_t)
            nc.vector.reciprocal(out=rstd, in_=rstd)
            zdt = F32R if tag == "ln1" else F32
            zt = io.tile([128, C], zdt, tag=f"z_{tag}")
            nc.vector.tensor_scalar(
                out=zt, in0=xt, scalar1=mv[:, 0:1], scalar2=rstd,
                op0=mybir.AluOpType.subtract, op1=mybir.AluOpType.mult,
            )
            return zt

        # phase A: LN1 + transpose into zT
        for i in range(NT):
            z1 = layer_norm_pre(xfull[:, i, :], "ln1")
            for k in range(NK):
                pt = ptr.tile([128, 128], F32R, tag="ptr", bufs=4)
                nc.tensor.transpose(pt, z1[:, k * 128:(k + 1) * 128], identr)
                nc.scalar.activation(
                    out=zT[:, k, i * 128:(i + 1) * 128], in_=pt, func=AF.Copy
                )

        rT = big.tile([128, NK, TLOC], F32R)
        vT = big.tile([128, NK, TLOC], F32R, tag="vT_wof", name="vT")

        # z1 (pre-affine LN1 output) exported for the launch-2 gate matmul
        z1b = io.tile([128, NK, TLOC], BF16, tag="z1b", bufs=1)
        nc.vector.tensor_copy(out=z1b, in_=zT)
        nc.sync.dma_start(out=z1t.rearrange("(k p) t -> p k t", p=128), in_=z1b)

        # phase B: the two z-consuming matmuls (r, v)
        for widx, wap in enumerate((wr, wv)):
            for m in range(NK):
                wtr = wpool.tile([128, NK, 128], F32R, tag="wchunkr")
                nc.sync.dma_start(out=wtr, in_=wap[m].rearrange("p (k c) -> p k c", c=128))
                for n in range(2):
                    ns = slice(n * 512, (n + 1) * 512)
                    ps = pmm.tile([128, 512], F32, tag="pmm")
                    for k in range(NK):
                        nc.tensor.matmul(
                            ps, wtr[:, k, :], zT[:, k, ns],
                            start=(k == 0), stop=(k == NK - 1),
                        )
                    bias_ap = btile[:, widx, m:m + 1]
                    if widx == 0:
                        nc.scalar.activation(
                            out=rT[:, m, ns], in_=ps, func=AF.Sigmoid, bias=bias_ap
                        )
                    else:
                        nc.vector.tensor_scalar_add(
                            out=vT[:, m, ns], in0=ps, scalar1=bias_ap
                        )

        # a = r * v (TF32, in place over rT)
        aT = rT
        for m in range(NK):
            nc.vector.tensor_mul(
                out=aT[:, m, :], in0=rT[:, m, :], in1=vT[:, m, :]
            )

        # att = a @ Wo ; preload all Wo chunks (slot shared with dead vT),
        # run n-outer so the first token half finishes early.
        attB = big.tile([128, NT, C], F32, tag="zT_attB")
        wof = big.tile([128, NK, NK, 128], F32R, tag="vT_wof", name="wof")
        for m in range(NK):
            nc.sync.dma_start(
                out=wof[:, m], in_=wo[m].rearrange("p (k c) -> p k c", c=128)
            )
        for n in range(2):
            ns = slice(n * 512, (n + 1) * 512)
            for m in range(NK):
                ps = pmm.tile([128, 512], F32, tag="pmm")
                for k in range(NK):
                    nc.tensor.matmul(
                        ps, wof[:, m, k, :], aT[:, k, ns],
                        start=(k == 0), stop=(k == NK - 1),
                    )
                attTm = io.tile([128, 512], F32, tag="attT_ev")
                nc.scalar.activation(out=attTm, in_=ps, func=AF.Copy)
                for j in range(4):
                    i_tok = n * 4 + j
                    pt = ptr.tile([128, 128], F32, tag="ptr", bufs=4)
                    nc.tensor.transpose(
                        pt, attTm[:, j * 128:(j + 1) * 128], ident
                    )
                    nc.any.tensor_copy(
                        out=attB[:, i_tok, m * 128:(m + 1) * 128], in_=pt
                    )
            # phase C for this token half: residual + LN2 pre-affine
            for i in range(n * 4, n * 4 + 4):
                x2t = io.tile([128, C], F32, tag="x2t")
                nc.vector.tensor_add(out=x2t, in0=xfull[:, i, :], in1=attB[:, i, :])
                nc.sync.dma_start(out=x2[i * 128:(i + 1) * 128, :], in_=x2t)
                z2t = layer_norm_pre(x2t, "ln2")
                nc.sync.dma_start(out=z2[i * 128:(i + 1) * 128, :], in_=z2t)

    nc.finalize()
    return nc


# ---------------------------------------------------------------- launch 2


def _build_launch2():
    nc = bacc.Bacc()
    # host-prepared layouts:
    #   htra/htrb/htt/sttp: [128, NK, CAP]  (p, k, t) = M[k*128+p, t]
    #   k2a/k2b: [NH, 128, C]  (hc, p, k*128+c) = K[k*128+p, hc*128+c]
    #   w1/w2: [NK, 128, C] chunk-lhsT;  w3: [128, NK, C]
    htra = nc.declare_dram_parameter("htra", [128, C // 128, CAP_A], BF16, isOutput=False)
    htrb = nc.declare_dram_parameter("htrb", [128, C // 128, CAP_B], BF16, isOutput=False)
    k2a = nc.declare_dram_parameter("k2a", [H // 128, 128, C], BF16, isOutput=False)
    k2b = nc.declare_dram_parameter("k2b", [H // 128, 128, C], BF16, isOutput=False)
    v2a = nc.declare_dram_parameter("v2a", [H, C], BF16, isOutput=False)
    v2b = nc.declare_dram_parameter("v2b", [H, C], BF16, isOutput=False)
    w1 = nc.declare_dram_parameter("w1", [C // 128, 128, C], BF16, isOutput=False)
    w2 = nc.declare_dram_parameter("w2", [C // 128, 128, C], BF16, isOutput=False)
    w3 = nc.declare_dram_parameter("w3", [128, C // 128, C], BF16, isOutput=False)
    htt = nc.declare_dram_parameter("htt", [128, C // 128, CAP_T], BF16, isOutput=False)
    z1tp = nc.declare_dram_parameter("z1tp", [128, C // 128, CAP_T], BF16, isOutput=False)
    bsr = nc.declare_dram_parameter("bsr", [C], F32, isOutput=False)
    outr = nc.declare_dram_parameter("outr", [CAP_R, C], F32, isOutput=True)
    outt = nc.declare_dram_parameter("outt", [CAP_T, C], F32, isOutput=True)

    NK = C // 128              # 8
    NH = H // 128              # 32

    with tile.TileContext(nc) as tc, ExitStack() as ctx:
        big = ctx.enter_context(tc.tile_pool(name="big", bufs=1))
        stream = ctx.enter_context(tc.tile_pool(name="stream", bufs=3))
        ev = ctx.enter_context(tc.tile_pool(name="ev", bufs=3))
        ps = ctx.enter_context(tc.tile_pool(name="ps", bufs=6, space="PSUM"))

        hTa = big.tile([128, NK, CAP_A], BF16)
        hTb = big.tile([128, NK, CAP_B], BF16)
        # hr: slot A tokens in [0, CAP_A), slot B in [CAP_A, CAP_R)
        hr = big.tile([128, NH, CAP_R], BF16)
        w3sb = big.tile([128, NK, C], BF16)

        nc.sync.dma_start(out=hTa, in_=htra[:])
        # R1 slot A
        for hc in range(NH):
            kt = stream.tile(
                [128, NK, 128], BF16, tag="kt0", name=f"kt_0_{hc}",
                bufs=6,
            )
            nc.sync.dma_start(
                out=kt, in_=k2a[hc].rearrange("p (k c) -> p k c", c=128)
            )
            pst = ps.tile([128, 512], F32, tag="ps", name=f"r1ps_0_{hc}")
            for k in range(NK):
                nc.tensor.matmul(
                    pst[:, :CAP_A], kt[:, k, :], hTa[:, k, :],
                    start=(k == 0), stop=(k == NK - 1),
                )
            rel = ev.tile([128, 512], F32, tag="rel")
            nc.vector.tensor_scalar_max(
                out=rel[:, :CAP_A], in0=pst[:, :CAP_A], scalar1=0.0
            )
            nc.vector.tensor_mul(
                out=hr[:, hc, 0:0 + CAP_A], in0=rel[:, :CAP_A], in1=rel[:, :CAP_A]
            )

        nc.sync.dma_start(out=hTb, in_=htrb[:])
        # T: transformer expert (state-gated), CAP_T tokens — runs first,
        # its small inputs load while K/V prefetch warms up behind it.
        hTt = big.tile([128, NK, CAP_T], BF16)
        nc.sync.dma_start(out=hTt, in_=htt[:])
        z1T = big.tile([128, NK, CAP_T], BF16)
        nc.sync.dma_start(out=z1T, in_=z1tp[:])
        bst = big.tile([128, NK], F32)
        nc.sync.dma_start(out=bst, in_=bsr.rearrange("(m p) -> p m", p=128))
        gT = big.tile([128, NK, CAP_T], BF16)

        for cc in range(NK):
            w1t = stream.tile([128, NK, 128], BF16, tag="w1t")
            nc.sync.dma_start(out=w1t, in_=w1[cc].rearrange("p (k c) -> p k c", c=128))
            psa = ps.tile([128, 512], F32, tag="pst", bufs=2)
            for k in range(NK):
                nc.tensor.matmul(
                    psa[:, :CAP_T], w1t[:, k, :], hTt[:, k, :],
                    start=(k == 0), stop=(k == NK - 1),
                )
            at = ev.tile([128, 512], F32, tag="at")
            nc.vector.tensor_copy(out=at[:, :CAP_T], in_=psa[:, :CAP_T])

            w2t = stream.tile([128, NK, 128], BF16, tag="w2t")
            nc.sync.dma_start(out=w2t, in_=w2[cc].rearrange("p (k c) -> p k c", c=128))
            psb = ps.tile([128, 512], F32, tag="pst", bufs=2)
            for k in range(NK):
                nc.tensor.matmul(
                    psb[:, :CAP_T], w2t[:, k, :], z1T[:, k, :],
                    start=(k == 0), stop=(k == NK - 1),
                )
            sg = ev.tile([128, 512], F32, tag="sg")
            nc.scalar.activation(
                out=sg[:, :CAP_T], in_=psb[:, :CAP_T], func=AF.Sigmoid,
                bias=bst[:, cc:cc + 1],
            )
            nc.vector.tensor_mul(
                out=gT[:, cc, :], in0=at[:, :CAP_T], in1=sg[:, :CAP_T]
            )

        nc.sync.dma_start(out=w3sb, in_=w3[:])
        tspans = [(0, 128), (128, 128), (256, 128), (384, CAP_T - 384)]
        for t0, tsz in tspans:
            for cn in range(2):
                pst = ps.tile(
                    [128, 512], F32, tag="pst", bufs=2, name=f"t3ps_{t0}_{cn}"
                )
                for k in range(NK):
                    nc.tensor.matmul(
                        pst[:tsz], gT[:, k, t0:t0 + tsz],
                        w3sb[:, k, cn * 512:(cn + 1) * 512],
                        start=(k == 0), stop=(k == NK - 1),
                    )
                oev = ev.tile([128, 512], F32, tag="oev", name=f"t3ev_{t0}_{cn}")
                nc.vector.tensor_copy(out=oev[:tsz], in_=pst[:tsz])
                nc.sync.dma_start(
                    out=outt[t0:t0 + tsz, cn * 512:(cn + 1) * 512], in_=oev[:tsz]
                )

        # R1 slot B
        for hc in range(NH):
            kt = stream.tile(
                [128, NK, 128], BF16, tag="ktCAP_A", name=f"kt_CAP_A_{hc}",
                bufs=6,
            )
            nc.sync.dma_start(
                out=kt, in_=k2b[hc].rearrange("p (k c) -> p k c", c=128)
            )
            pst = ps.tile([128, 512], F32, tag="ps", name=f"r1ps_CAP_A_{hc}")
            for k in range(NK):
                nc.tensor.matmul(
                    pst[:, :CAP_B], kt[:, k, :], hTb[:, k, :],
                    start=(k == 0), stop=(k == NK - 1),
                )
            rel = ev.tile([128, 512], F32, tag="rel")
            nc.vector.tensor_scalar_max(
                out=rel[:, :CAP_B], in0=pst[:, :CAP_B], scalar1=0.0
            )
            nc.vector.tensor_mul(
                out=hr[:, hc, CAP_A:CAP_A + CAP_B], in0=rel[:, :CAP_B], in1=rel[:, :CAP_B]
            )

        # R2: out_r = hr^T @ V, tokens as M (token-major out).
        # token tiles 0-2 belong to slot A (v2a), tiles 3-4 to slot B (v2b).
        for tiles, v2s in (((0, 1, 2), v2a), ((3, 4), v2b)):
            psts = {}
            for tt in tiles:
                for cn in range(2):
                    psts[tt, cn] = ps.tile(
                        [128, 512], F32, tag="ps", name=f"r2ps_{tt}_{cn}"
                    )
            for hc in range(NH):
                vt = stream.tile(
                    [128, C], BF16, tag="vt", name=f"vt_{hc}", bufs=8
                )
                nc.sync.dma_start(out=vt, in_=v2s[hc * 128:(hc + 1) * 128, :])
                for tt in tiles:
                    t0 = tt * 128
                    for cn in range(2):
                        nc.tensor.matmul(
                            psts[tt, cn],
                            hr[:, hc, t0:t0 + 128],
                            vt[:, cn * 512:(cn + 1) * 512],
                            start=(hc == 0), stop=(hc == NH - 1),
                            skip_group_check=True,
                        )
            for tt in tiles:
                t0 = tt * 128
                for cn in range(2):
                    oev = ev.tile([128, 512], F32, tag="oev", name=f"oev_{tt}_{cn}")
                    nc.vector.tensor_copy(out=oev, in_=psts[tt, cn])
                    nc.sync.dma_start(
                        out=outr[t0:t0 + 128, cn * 512:(cn + 1) * 512], in_=oev
                    )

    nc.finalize()
    return nc


def _get_programs(zero_b):
    key1 = f"nc1_{zero_b}"
    if key1 not in _CACHE:
        _CACHE[key1] = _build_launch1(zero_b)
    if "nc2" not in _CACHE:
        _CACHE["nc2"] = _build_launch2()
    return _CACHE[key1], _CACHE["nc2"]


# ---------------------------------------------------------------- host math


def _sigmoid(x):
    return 1.0 / (1.0 + np.exp(-x.astype(np.float32), dtype=np.float32))


def _ln_np(x, w, b):
    x = x.astype(np.float32)
    m = x.mean(axis=-1, keepdims=True, dtype=np.float32)
    v = x.var(axis=-1, keepdims=True, dtype=np.float32)
    return ((x - m) / np.sqrt(v + np.float32(LN_EPS)) * w + b).astype(np.float32)


def _expert_out_host(hrows, strows, wvec, K_rwkv, V_rwkv, W1, W2, W3):
    """Exact fp32 expert outputs for a small token batch (reference order)."""
    out = np.zeros((hrows.shape[0], C), np.float32)
    for e in (0, 1):
        m = wvec == e
        if m.any():
            z = hrows[m] @ K_rwkv[e]
            hr = np.square(np.maximum(z, 0.0))
            out[m] = hr @ V_rwkv[e]
    m = wvec == 2
    if m.any():
        out[m] = ((hrows[m] @ W1) * _sigmoid(strows[m] @ W2)) @ W3
    return out


def _routing_from_h(h, inp):
    """bids (N,3) in reference op order."""
    Wcat = np.concatenate(
        [
            np.asarray(inp["conf_rwkv"], np.float32).T,
            np.asarray(inp["conf_trans"], np.float32)[:, None],
            np.asarray(inp["w_diff"], np.float32)[:, None],
            np.asarray(inp["W_aff"], np.float32),
        ],
        axis=1,
    )
    Q = h @ Wcat
    conf = _sigmoid(Q[:, 0:3])
    diff = _sigmoid(Q[:, 3])
    cap = np.asarray(inp["capital_shares"], np.float32)
    bids = conf * cap[None, :] * diff[:, None]
    bids = bids + Q[:, 4:7]
    return bids, conf


def _tf32_round(a):
    """Round fp32 to TF32 (10-bit mantissa, round-to-nearest-even)."""
    u = np.ascontiguousarray(a, np.float32).view(np.uint32)
    r = (u + np.uint32(0xFFF) + ((u >> np.uint32(13)) & np.uint32(1))) & np.uint32(
        0xFFFFE000
    )
    return r.view(np.float32)


# ---------------------------------------------------------------- kernel


def kernel(**inputs):
    x = np.ascontiguousarray(np.asarray(inputs["x"], np.float32))
    assert x.shape == (B, T, C), x.shape
    ln1w = np.asarray(inputs["ln1_w"], np.float32)
    ln1b = np.asarray(inputs["ln1_b"], np.float32)
    ln2w = np.asarray(inputs["ln2_w"], np.float32)
    ln2b = np.asarray(inputs["ln2_b"], np.float32)
    Wr = np.asarray(inputs["Wr"], np.float32)
    Wv = np.asarray(inputs["Wv"], np.float32)
    Wo = np.asarray(inputs["Wo"], np.float32)
    Ws = np.asarray(inputs["Ws"], np.float32)
    K_rwkv = np.asarray(inputs["K_rwkv"], np.float32)
    V_rwkv = np.asarray(inputs["V_rwkv"], np.float32)
    W1 = np.asarray(inputs["W1"], np.float32)
    W2 = np.asarray(inputs["W2"], np.float32)
    W3 = np.asarray(inputs["W3"], np.float32)

    trace = _trace_enabled()
    if trace:
        _install_trace_shims()
        LAST_EXEC_NS.clear()

    nc1, nc2 = _get_programs(zero_b=not np.any(ln1b))
    xf = x.reshape(N, C)

    # ---- launch 1
    def _chunk_l1(W):
        # [m, p, k*128+c] with element W[k*128+p, m*128+c]
        return np.ascontiguousarray(
            W.reshape(8, 128, 8, 128).transpose(2, 1, 0, 3).reshape(8, 128, C)
        )

    wrp = _tf32_round(_chunk_l1(ln1w[:, None] * Wr))
    wvp = _tf32_round(_chunk_l1(ln1w[:, None] * Wv))
    wod = _tf32_round(_chunk_l1(Wo))
    brows = np.ascontiguousarray(
        np.stack([ln1b @ Wr, ln1b @ Wv, ln1b @ Ws]).astype(np.float32)
    )
    in1 = [
        {
            "x": xf[c * TLOC:(c + 1) * TLOC],
            "wr": wrp, "wv": wvp,
            "wo": wod, "brows": brows,
        }
        for c in range(NCORES)
    ]
    res1 = run_bass_kernel_spmd(nc1, in1, list(range(NCORES)), trace=trace)
    if trace:
        LAST_EXEC_NS.append(res1.exec_time_ns)
    x2 = np.concatenate([res1.results[c]["x2"] for c in range(NCORES)], axis=0)
    z2 = np.concatenate([res1.results[c]["z2"] for c in range(NCORES)], axis=0)
    z1T = np.concatenate([res1.results[c]["z1t"] for c in range(NCORES)], axis=1)

    # ---- host routing
    h = z2 * ln2w + ln2b
    bids, conf = _routing_from_h(h, inputs)
    order = np.argsort(bids, axis=1)
    winners = order[:, 2].astype(np.int64)
    gap = np.take_along_axis(bids, order[:, 2:3], 1)[:, 0] - np.take_along_axis(
        bids, order[:, 1:2], 1
    )[:, 0]
    margin_idx = np.nonzero(gap < MARGIN)[0]

    # exact recompute of borderline tokens (fp32, reference order)
    exact = {}
    if margin_idx.size:
        xr = xf[margin_idx]
        xln = _ln_np(xr, ln1w, ln1b)
        att = (_sigmoid(xln @ Wr) * (xln @ Wv)) @ Wo
        x2e = xr + att
        he = _ln_np(x2e, ln2w, ln2b)
        ste = xln @ Ws
        bide, confe = _routing_from_h(he, inputs)
        we = np.argmax(bide, axis=1)
        wce = np.take_along_axis(confe, we[:, None], 1)[:, 0]
        sce = wce / (wce + np.float32(1e-6))
        oute = _expert_out_host(he, ste, we, K_rwkv, V_rwkv, W1, W2, W3)
        for j, t in enumerate(margin_idx):
            exact[int(t)] = x2e[j] + oute[j] * sce[j]

    win_conf = np.take_along_axis(conf, winners[:, None], 1)[:, 0]
    scale = win_conf / (win_conf + np.float32(1e-6))

    # ---- pack tokens for launch 2
    is_margin = np.zeros(N, bool)
    is_margin[margin_idx] = True
    host_extra = []  # (token, winner) computed on host

    # 16 rwkv slots: per core one A slot (CAP_A) and one B slot (CAP_B);
    # each slot carries its own K/V, so any slot can serve either expert.
    # Greedy largest-first bin packing, leftovers go to the host.
    avail = [(c, "a", CAP_A) for c in range(NCORES)] + [
        (c, "b", CAP_B) for c in range(NCORES)
    ]
    slot_assign = {}  # (core, "a"/"b") -> (idx, expert)
    counts = [np.nonzero((winners == e) & ~is_margin)[0] for e in (0, 1)]
    for e in sorted((0, 1), key=lambda e: -counts[e].size):
        idx = counts[e]
        pos = 0
        while pos < idx.size and avail:
            avail.sort(key=lambda t: -t[2])
            c, ab, cap = avail.pop(0)
            take = min(cap, idx.size - pos)
            slot_assign[(c, ab)] = (idx[pos:pos + take], e)
            pos += take
        if pos < idx.size:
            host_extra.extend((int(t), e) for t in idx[pos:])

    idx_t = np.nonzero((winners == 2) & ~is_margin)[0]
    if idx_t.size > NCORES * CAP_T:
        host_extra.extend((int(t), 2) for t in idx_t[NCORES * CAP_T:])
        idx_t = idx_t[:NCORES * CAP_T]
    per = (idx_t.size + NCORES - 1) // NCORES if idx_t.size else 0
    core_t = [idx_t[c * per:(c + 1) * per] for c in range(NCORES)]

    hbf = h.astype(BF16_NP)
    def _chunk_l2(W):
        # [m, p, k*128+c] bf16 chunk-lhsT layout
        return np.ascontiguousarray(
            W.reshape(8, 128, 8, 128).transpose(2, 1, 0, 3).reshape(8, 128, C)
        ).astype(BF16_NP)

    k_bf = {
        e: np.ascontiguousarray(
            K_rwkv[e].reshape(8, 128, 32, 128).transpose(2, 1, 0, 3).reshape(32, 128, C)
        ).astype(BF16_NP)
        for e in (0, 1)
    }
    v_bf = {e: np.ascontiguousarray(V_rwkv[e]).astype(BF16_NP) for e in (0, 1)}
    w1c = _chunk_l2(W1)
    w2c = _chunk_l2((ln1w[:, None] * Ws) @ W2)
    w3b = np.ascontiguousarray(
        W3.reshape(8, 128, C).transpose(1, 0, 2)
    ).astype(BF16_NP)
    bsrow = np.ascontiguousarray((ln1b @ Ws @ W2).astype(np.float32))

    def _pack_T(mat_cols, cap):
        # [C, cnt] -> [128, 8, cap] with (p, k, t) = mat[k*128+p, t]
        out = np.zeros((128, 8, cap), BF16_NP)
        cnt = mat_cols.shape[1]
        if cnt:
            out[:, :, :cnt] = mat_cols.reshape(8, 128, cnt).transpose(1, 0, 2)
        return out

    empty = np.empty(0, np.int64)
    in2 = []
    for c in range(NCORES):
        idx_a, ea = slot_assign.get((c, "a"), (empty, 0))
        idx_b, eb = slot_assign.get((c, "b"), (empty, 0))
        ti = core_t[c]
        in2.append(
            {
                "htra": _pack_T(np.ascontiguousarray(hbf[idx_a].T), CAP_A),
                "htrb": _pack_T(np.ascontiguousarray(hbf[idx_b].T), CAP_B),
                "k2a": k_bf[ea], "v2a": v_bf[ea],
                "k2b": k_bf[eb], "v2b": v_bf[eb],
                "w1": w1c, "w2": w2c, "w3": w3b,
                "bsr": bsrow,
                "htt": _pack_T(np.ascontiguousarray(hbf[ti].T), CAP_T),
                "z1tp": _pack_T(np.ascontiguousarray(z1T[:, ti]), CAP_T),
            }
        )
    res2 = run_bass_kernel_spmd(nc2, in2, list(range(NCORES)), trace=trace)
    if trace:
        LAST_EXEC_NS.append(res2.exec_time_ns)

    # ---- combine
    y = x2.copy()
    empty = np.empty(0, np.int64)
    for c in range(NCORES):
        outr_c = res2.results[c]["outr"]
        idx_a, _ = slot_assign.get((c, "a"), (empty, 0))
        if idx_a.size:
            y[idx_a] += outr_c[:idx_a.size] * scale[idx_a, None]
        idx_b, _ = slot_assign.get((c, "b"), (empty, 0))
        if idx_b.size:
            y[idx_b] += (
                outr_c[CAP_A:CAP_A + idx_b.size] * scale[idx_b, None]
            )
        ti = core_t[c]
        if ti.size:
            y[ti] += res2.results[c]["outt"][:ti.size] * scale[ti, None]

    if host_extra:
        toks = np.array([t for t, _ in host_extra], np.int64)
        wv_ = winners[toks]
        xln_rows = z1T[:, toks].T.astype(np.float32) * ln1w + ln1b
        st_rows = xln_rows @ Ws
        out_h = _expert_out_host(
            h[toks], st_rows, wv_, K_rwkv, V_rwkv, W1, W2, W3
        )
        y[toks] += out_h * scale[toks, None]

    for t, row in exact.items():
        y[t] = row

    return np.ascontiguousarray(y.reshape(B, T, C).astype(np.float32))



# revision 10
# speedup vs baseline: 1.1384x; 1.1384x over previous
"""Trainium2 Bass kernel for nn_CaMoE_Block (MoE routing block).

Strategy (8 NeuronCores, 2 launches, host routing between):
  Launch 1 -- data-parallel over tokens (8192/8 = 1024 per core), all bf16:
    LN1 (pre-affine, ln1_w folded into weights) -> DMA-XBAR transpose ->
    gated attention matmuls in [token, C] output layout (z stationary,
    weights moving; 512-wide moving rows) -> residual x2 = x + att.
    Exports ONLY x2 (f32); host recomputes LN2/z1 from x/x2.
  Host -- routing: h = LN2(x2)*w+b, bids, winners; borderline tokens
    (top-2 gap < MARGIN) recomputed exactly in fp32 reference order.
    Packing: one rwkv expert per core (5 cores e0 / 3 cores e1 for the
    reference distribution); overflow tokens computed exactly on host.
  Launch 2 -- expert-parallel: each core gets ONE rwkv expert's K/V plus
    the shared transformer-expert weights. R1 (squared-relu K matmul,
    640-token slot) -> transformer expert (416-token slot) -> R2 (V
    matmul, V resident in SBUF). Outputs bf16.
  Host -- scale by straight-through confidence, scatter-add residual.
"""

import os
import sys

sys.path.insert(0, "/opt/trn_rl_repo")

from contextlib import ExitStack

import ml_dtypes
import numpy as np

import concourse.bacc as bacc
import concourse.tile as tile
from concourse import mybir
from concourse.bass_utils import run_bass_kernel_spmd
from concourse.masks import make_identity

F32 = mybir.dt.float32
BF16 = mybir.dt.bfloat16
BF16_NP = ml_dtypes.bfloat16
AF = mybir.ActivationFunctionType
ALU = mybir.AluOpType

B, T, C = 4, 2048, 1024
N = B * T                      # 8192 tokens
NCORES = 8
TLOC = N // NCORES             # 1024 tokens per core
H = 4 * C                      # 4096
NK = C // 128                  # 8
NT = TLOC // 128               # 8
NH = H // 128                  # 32
CAP_R = 640                    # rwkv tokens per core in launch 2
CAP_T = 416                    # transformer tokens per core in launch 2
MARGIN = 3e-3                  # top-2 bid gap below which host recomputes
LN_EPS = 1e-5

# populated when BASS_MOE_TRACE=1: [launch1_ns, launch2_ns]
LAST_EXEC_NS = []

_CACHE = {}


def _trace_enabled():
    return bool(int(os.environ.get("BASS_MOE_TRACE", "0")))


def _install_trace_shims():
    """This image lacks antenv.axon_hooks; synthesize it so trace=True works."""
    import types

    import antenv
    import concourse.bass_utils as bass_utils

    if "antenv.axon_hooks" not in sys.modules:
        from trn_agent_boot.trn_boot import _ntff_profile_via_ctypes

        mod = types.ModuleType("antenv.axon_hooks")
        hook = _ntff_profile_via_ctypes("/opt/axon/libaxon_pjrt.so")
        mod.get_axon_ntff_profile_hook = lambda: hook
        mod.set_axon_ntff_profile_hook = lambda h: None
        sys.modules["antenv.axon_hooks"] = mod
        antenv.axon_hooks = mod
    bass_utils.upload_artifacts = lambda tmpdir: "local://" + tmpdir


# ---------------------------------------------------------------- launch 1


def _build_launch1(with_bias):
    nc = bacc.Bacc()
    x = nc.declare_dram_parameter("x", [TLOC, C], F32, isOutput=False)
    # weights: [p, kc, c] with element W'[kc*128+p, c]  (W' = ln1w-folded)
    wr = nc.declare_dram_parameter("wr", [128, NK, C], BF16, isOutput=False)
    wv = nc.declare_dram_parameter("wv", [128, NK, C], BF16, isOutput=False)
    wo = nc.declare_dram_parameter("wo", [128, NK, C], BF16, isOutput=False)
    if with_bias:
        # rows b@Wr', b@Wv' as [1, C] each
        brow = nc.declare_dram_parameter("brow", [2, C], BF16, isOutput=False)
    x2 = nc.declare_dram_parameter("x2", [TLOC, C], F32, isOutput=True)

    with tile.TileContext(nc) as tc, ExitStack() as ctx:
        big = ctx.enter_context(tc.tile_pool(name="big", bufs=1))
        io = ctx.enter_context(tc.tile_pool(name="io", bufs=3))
        stat = ctx.enter_context(tc.tile_pool(name="stat", bufs=4))
        prv = ctx.enter_context(tc.tile_pool(name="prv", bufs=4, space="PSUM"))
        patt = ctx.enter_context(tc.tile_pool(name="patt", bufs=4, space="PSUM"))

        eps_t = big.tile([128, 1], F32)
        nc.vector.memset(eps_t, LN_EPS)

        # per-queue DMA bandwidth is low (~22GB/s): split critical-path
        # transfers into small chunks spread across the 16 queues.
        xfull = big.tile([128, NT, C], F32)
        xr_ap = x.rearrange("(i p) c -> p i c", p=128)
        for j in range(4):
            cs = slice(j * 256, (j + 1) * 256)
            nc.sync.dma_start(out=xfull[:, 0, cs], in_=xr_ap[:, 0, cs])

        # weights staged whole; wr/wv first (needed first), wo behind
        wrS = big.tile([128, NK, C], BF16)
        wvS = big.tile([128, NK, C], BF16)
        woS = big.tile([128, NK, C], BF16)
        for k in range(NK):
            for half in range(2):
                hs = slice(half * 512, (half + 1) * 512)
                nc.sync.dma_start(out=wrS[:, k, hs], in_=wr[:, k, hs])
                nc.sync.dma_start(out=wvS[:, k, hs], in_=wv[:, k, hs])
        for i in range(1, NT):
            for j in range(2):
                cs = slice(j * 512, (j + 1) * 512)
                nc.sync.dma_start(out=xfull[:, i, cs], in_=xr_ap[:, i, cs])
        for k in range(NK):
            nc.sync.dma_start(out=woS[:, k, :], in_=wo[:, k, :])
        if with_bias:
            brS = big.tile([128, 2, C], BF16)
            nc.sync.dma_start(
                out=brS[0:1], in_=brow.rearrange("b c -> 1 b c")
            )
            onesS = big.tile([1, 128], BF16)
            nc.vector.memset(onesS, 1.0)

        def layer_norm_pre(xt, tag):
            """-> z = (x - mean) * rstd as a fresh [128, C] bf16 tile."""
            stats = stat.tile([128, 2, 6], F32, tag=f"st_{tag}")
            nc.vector.bn_stats(out=stats[:, 0, :], in_=xt[:, 0:512])
            nc.vector.bn_stats(out=stats[:, 1, :], in_=xt[:, 512:1024])
            mv = stat.tile([128, 2], F32, tag=f"mv_{tag}")
            nc.vector.bn_aggr(out=mv, in_=stats)
            rstd = stat.tile([128, 1], F32, tag=f"rs_{tag}")
            nc.scalar.activation(out=rstd, in_=mv[:, 1:2], func=AF.Sqrt, bias=eps_t)
            nc.vector.reciprocal(out=rstd, in_=rstd)
            zt = io.tile([128, C], BF16, tag=f"z_{tag}")
            nc.vector.tensor_scalar(
                out=zt, in0=xt, scalar1=mv[:, 0:1], scalar2=rstd,
                op0=ALU.subtract, op1=ALU.mult,
            )
            return zt

        # software pipeline over token tiles:
        #   stage A(i): LN1 -> zT via XBAR transpose -> r/v matmuls
        #   stage B(i): evict r,v -> a=r*v -> aT -> att matmul -> x2
        # PE order: A0 A1 B0 A2 B1 ... A7 B6 B7
        pr = {}
        po = {}
        amul = {}

        def stage_a(i):
            zt = layer_norm_pre(xfull[:, i, :], "ln1")
            zT = io.tile([128, NK, 128], BF16, tag="zT")
            nc.sync.dma_start_transpose(out=zT, in_=zt)
            ps = [
                prv.tile([128, 512], F32, tag="prv", name=f"prv_{i}_{j}")
                for j in range(4)
            ]
            for k in range(NK):
                st = dict(start=(k == 0), stop=(k == NK - 1 and not with_bias))
                nc.tensor.matmul(
                    ps[0], zT[:, k, :], wrS[:, k, 0:512],
                    skip_group_check=True, **st,
                )
                nc.tensor.matmul(
                    ps[1], zT[:, k, :], wrS[:, k, 512:1024],
                    skip_group_check=True, **st,
                )
                nc.tensor.matmul(
                    ps[2], zT[:, k, :], wvS[:, k, 0:512],
                    skip_group_check=True, **st,
                )
                nc.tensor.matmul(
                    ps[3], zT[:, k, :], wvS[:, k, 512:1024],
                    skip_group_check=True, **st,
                )
            if with_bias:
                for j in range(4):
                    nc.tensor.matmul(
                        ps[j], onesS,
                        brS[0:1, j // 2, (j % 2) * 512:(j % 2 + 1) * 512],
                        start=False, stop=True, skip_group_check=True,
                    )
            pr[i] = ps

        def stage_b(i):
            ps = pr.pop(i)
            rv = io.tile([128, 2, C], BF16, tag="rv")
            nc.scalar.activation(out=rv[:, 0, 0:512], in_=ps[0], func=AF.Sigmoid)
            nc.scalar.activation(out=rv[:, 0, 512:1024], in_=ps[1], func=AF.Sigmoid)
            nc.vector.tensor_copy(out=rv[:, 1, 0:512], in_=ps[2])
            nc.vector.tensor_copy(out=rv[:, 1, 512:1024], in_=ps[3])
            at = io.tile([128, C], BF16, tag="amul")
            nc.vector.tensor_mul(out=at, in0=rv[:, 0, :], in1=rv[:, 1, :])
            aT = io.tile([128, NK, 128], BF16, tag="aT")
            nc.sync.dma_start_transpose(out=aT, in_=at)
            pa = [
                patt.tile([128, 512], F32, tag="patt", name=f"patt_{i}_{j}")
                for j in range(2)
            ]
            for k in range(NK):
                st = dict(start=(k == 0), stop=(k == NK - 1))
                nc.tensor.matmul(
                    pa[0], aT[:, k, :], woS[:, k, 0:512],
                    skip_group_check=True, **st,
                )
                nc.tensor.matmul(
                    pa[1], aT[:, k, :], woS[:, k, 512:1024],
                    skip_group_check=True, **st,
                )
            x2t = io.tile([128, C], F32, tag="x2t")
            nc.vector.tensor_add(out=x2t[:, 0:512], in0=xfull[:, i, 0:512], in1=pa[0])
            nc.sync.dma_start(
                out=x2[i * 128:(i + 1) * 128, 0:512], in_=x2t[:, 0:512]
            )
            nc.vector.tensor_add(
                out=x2t[:, 512:1024], in0=xfull[:, i, 512:1024], in1=pa[1]
            )
            nc.sync.dma_start(
                out=x2[i * 128:(i + 1) * 128, 512:1024], in_=x2t[:, 512:1024]
            )

        stage_a(0)
        for i in range(1, NT):
            stage_a(i)
            stage_b(i - 1)
        stage_b(NT - 1)

    nc.finalize()
    return nc


# ---------------------------------------------------------------- launch 2


def _build_launch2():
    nc = bacc.Bacc()
    # host-prepared layouts:
    #   ht/htt/z1tp: [128, NK, CAP]  (p, k, t) = M[k*128+p, t]
    #   k2: [NH, 128, C]  (hc, p, k*128+c) = K[k*128+p, hc*128+c]
    #   w1/w2: [NK, 128, C] chunk-lhsT;  w3: [128, NK, C]
    #   v2: [H, C]
    ht = nc.declare_dram_parameter("ht", [128, NK, CAP_R], BF16, isOutput=False)
    k2 = nc.declare_dram_parameter("k2", [NH, 128, C], BF16, isOutput=False)
    v2 = nc.declare_dram_parameter("v2", [H, C], BF16, isOutput=False)
    w1 = nc.declare_dram_parameter("w1", [NK, 128, C], BF16, isOutput=False)
    w2 = nc.declare_dram_parameter("w2", [NK, 128, C], BF16, isOutput=False)
    w3 = nc.declare_dram_parameter("w3", [128, NK, C], BF16, isOutput=False)
    htt = nc.declare_dram_parameter("htt", [128, NK, CAP_T], BF16, isOutput=False)
    z1tp = nc.declare_dram_parameter("z1tp", [128, NK, CAP_T], BF16, isOutput=False)
    bsr = nc.declare_dram_parameter("bsr", [C], F32, isOutput=False)
    outr = nc.declare_dram_parameter("outr", [CAP_R, C], BF16, isOutput=True)
    outt = nc.declare_dram_parameter("outt", [CAP_T, C], BF16, isOutput=True)

    CR0 = 512                  # R1 psum split: 512 + 128
    CR1 = CAP_R - CR0

    with tile.TileContext(nc) as tc, ExitStack() as ctx:
        big = ctx.enter_context(tc.tile_pool(name="big", bufs=1))
        stream = ctx.enter_context(tc.tile_pool(name="stream", bufs=6))
        wst = ctx.enter_context(tc.tile_pool(name="wst", bufs=4))
        ev = ctx.enter_context(tc.tile_pool(name="ev", bufs=3))
        ps = ctx.enter_context(tc.tile_pool(name="ps", bufs=6, space="PSUM"))

        hT = big.tile([128, NK, CAP_R], BF16)
        hr = big.tile([128, NH, CAP_R], BF16)
        v2sb = big.tile([128, NH, C], BF16)
        w3sb = big.tile([128, NK, C], BF16)
        hTt = big.tile([128, NK, CAP_T], BF16)
        z1T = big.tile([128, NK, CAP_T], BF16)
        gT = big.tile([128, NK, CAP_T], BF16)
        bst = big.tile([128, NK], F32)

        # ht needed first: one chunk per k-slice across queues
        for k in range(NK):
            nc.sync.dma_start(out=hT[:, k, :], in_=ht[:, k, :])
        # prefetches (issued up front; consumed later)
        nc.sync.dma_start(out=hTt, in_=htt[:])
        nc.sync.dma_start(out=z1T, in_=z1tp[:])
        nc.sync.dma_start(out=bst, in_=bsr.rearrange("(m p) -> p m", p=128))
        v2r = v2.rearrange("(hc p) c -> p hc c", p=128)
        for g in range(8):
            nc.sync.dma_start(
                out=v2sb[:, g * 4:(g + 1) * 4, :], in_=v2r[:, g * 4:(g + 1) * 4, :]
            )
        for g in range(2):
            nc.sync.dma_start(
                out=w3sb[:, g * 4:(g + 1) * 4, :], in_=w3[:, g * 4:(g + 1) * 4, :]
            )

        # ---- R1: hr = relu(h @ K)^2, output [hc, token] layout
        for hc in range(NH):
            kt = stream.tile([128, NK, 128], BF16, tag="kt", name=f"kt_{hc}")
            k2r = k2[hc].rearrange("p (k c) -> p k c", c=128)
            nc.sync.dma_start(out=kt[:, 0:4, :], in_=k2r[:, 0:4, :])
            nc.sync.dma_start(out=kt[:, 4:8, :], in_=k2r[:, 4:8, :])
            pa = ps.tile([128, 512], F32, tag="ps", name=f"r1a_{hc}")
            pb = ps.tile([128, 512], F32, tag="ps", name=f"r1b_{hc}")
            for k in range(NK):
                st = dict(start=(k == 0), stop=(k == NK - 1))
                nc.tensor.matmul(
                    pa, kt[:, k, :], hT[:, k, 0:CR0], skip_group_check=True, **st
                )
                nc.tensor.matmul(
                    pb[:, 0:CR1], kt[:, k, :], hT[:, k, CR0:CAP_R],
                    skip_group_check=True, **st,
                )
            rel = ev.tile([128, CAP_R], F32, tag="rel")
            nc.scalar.activation(out=rel[:, 0:CR0], in_=pa, func=AF.Relu)
            nc.scalar.activation(out=rel[:, CR0:CAP_R], in_=pb[:, 0:CR1], func=AF.Relu)
            nc.vector.tensor_mul(
                out=hr[:, hc, 0:CR0], in0=rel[:, 0:CR0], in1=rel[:, 0:CR0]
            )
            nc.vector.tensor_mul(
                out=hr[:, hc, CR0:CAP_R], in0=rel[:, CR0:CAP_R],
                in1=rel[:, CR0:CAP_R],
            )

        # ---- T: transformer expert (state-gated)
        for cc in range(NK):
            w1t = wst.tile([128, NK, 128], BF16, tag="w1t")
            w1r = w1[cc].rearrange("p (k c) -> p k c", c=128)
            nc.sync.dma_start(out=w1t[:, 0:4, :], in_=w1r[:, 0:4, :])
            nc.sync.dma_start(out=w1t[:, 4:8, :], in_=w1r[:, 4:8, :])
            psa = ps.tile([128, 512], F32, tag="pst", bufs=2, name=f"ta_{cc}")
            for k in range(NK):
                nc.tensor.matmul(
                    psa[:, :CAP_T], w1t[:, k, :], hTt[:, k, :],
                    start=(k == 0), stop=(k == NK - 1), skip_group_check=True,
                )
            at = ev.tile([128, 512], F32, tag="at")
            nc.vector.tensor_copy(out=at[:, :CAP_T], in_=psa[:, :CAP_T])

            w2t = wst.tile([128, NK, 128], BF16, tag="w2t")
            w2r = w2[cc].rearrange("p (k c) -> p k c", c=128)
            nc.sync.dma_start(out=w2t[:, 0:4, :], in_=w2r[:, 0:4, :])
            nc.sync.dma_start(out=w2t[:, 4:8, :], in_=w2r[:, 4:8, :])
            psb = ps.tile([128, 512], F32, tag="pst", bufs=2, name=f"tg_{cc}")
            for k in range(NK):
                nc.tensor.matmul(
                    psb[:, :CAP_T], w2t[:, k, :], z1T[:, k, :],
                    start=(k == 0), stop=(k == NK - 1), skip_group_check=True,
                )
            sg = ev.tile([128, 512], F32, tag="sg")
            nc.scalar.activation(
                out=sg[:, :CAP_T], in_=psb[:, :CAP_T], func=AF.Sigmoid,
                bias=bst[:, cc:cc + 1],
            )
            nc.vector.tensor_mul(
                out=gT[:, cc, :], in0=at[:, :CAP_T], in1=sg[:, :CAP_T]
            )

        tspans = [(0, 128), (128, 128), (256, 128), (384, CAP_T - 384)]
        for t0, tsz in tspans:
            for cn in range(2):
                pst = ps.tile([128, 512], F32, tag="pst", bufs=2, name=f"t3_{t0}_{cn}")
                for k in range(NK):
                    nc.tensor.matmul(
                        pst[:tsz], gT[:, k, t0:t0 + tsz],
                        w3sb[:, k, cn * 512:(cn + 1) * 512],
                        start=(k == 0), stop=(k == NK - 1), skip_group_check=True,
                    )
                oev = ev.tile([128, 512], BF16, tag="oev", name=f"t3ev_{t0}_{cn}")
                nc.vector.tensor_copy(out=oev[:tsz], in_=pst[:tsz])
                nc.sync.dma_start(
                    out=outt[t0:t0 + tsz, cn * 512:(cn + 1) * 512], in_=oev[:tsz]
                )

        # ---- R2: out_r = hr^T @ V, V resident; two cn passes of 5 banks
        for cn in range(2):
            psts = [
                ps.tile([128, 512], F32, tag="ps", name=f"r2_{cn}_{tt}")
                for tt in range(5)
            ]
            for hc in range(NH):
                for tt in range(5):
                    nc.tensor.matmul(
                        psts[tt], hr[:, hc, tt * 128:(tt + 1) * 128],
                        v2sb[:, hc, cn * 512:(cn + 1) * 512],
                        start=(hc == 0), stop=(hc == NH - 1),
                        skip_group_check=True,
                    )
            for tt in range(5):
                oev = ev.tile([128, 512], BF16, tag="oev", name=f"r2ev_{cn}_{tt}")
                nc.vector.tensor_copy(out=oev, in_=psts[tt])
                nc.sync.dma_start(
                    out=outr[tt * 128:(tt + 1) * 128, cn * 512:(cn + 1) * 512],
                    in_=oev,
                )

    nc.finalize()
    return nc


def _get_programs(with_bias):
    key1 = f"nc1_{with_bias}"
    if key1 not in _CACHE:
        _CACHE[key1] = _build_launch1(with_bias)
    if "nc2" not in _CACHE:
        _CACHE["nc2"] = _build_launch2()
    return _CACHE[key1], _CACHE["nc2"]


# ---------------------------------------------------------------- host math


def _sigmoid(x):
    return 1.0 / (1.0 + np.exp(-x.astype(np.float32), dtype=np.float32))


def _ln_np(x, w, b):
    x = x.astype(np.float32)
    m = x.mean(axis=-1, keepdims=True, dtype=np.float32)
    v = x.var(axis=-1, keepdims=True, dtype=np.float32)
    return ((x - m) / np.sqrt(v + np.float32(LN_EPS)) * w + b).astype(np.float32)


def _ln_pre_np(x):
    x = x.astype(np.float32)
    m = x.mean(axis=-1, keepdims=True, dtype=np.float32)
    v = x.var(axis=-1, keepdims=True, dtype=np.float32)
    return ((x - m) / np.sqrt(v + np.float32(LN_EPS))).astype(np.float32)


def _expert_out_host(hrows, strows, wvec, K_rwkv, V_rwkv, W1, W2, W3):
    """Exact fp32 expert outputs for a small token batch (reference order)."""
    out = np.zeros((hrows.shape[0], C), np.float32)
    for e in (0, 1):
        m = wvec == e
        if m.any():
            z = hrows[m] @ K_rwkv[e]
            hr = np.square(np.maximum(z, 0.0))
            out[m] = hr @ V_rwkv[e]
    m = wvec == 2
    if m.any():
        out[m] = ((hrows[m] @ W1) * _sigmoid(strows[m] @ W2)) @ W3
    return out


def _routing_from_h(h, inp):
    """bids (N,3) in reference op order."""
    Wcat = np.concatenate(
        [
            np.asarray(inp["conf_rwkv"], np.float32).T,
            np.asarray(inp["conf_trans"], np.float32)[:, None],
            np.asarray(inp["w_diff"], np.float32)[:, None],
            np.asarray(inp["W_aff"], np.float32),
        ],
        axis=1,
    )
    Q = h @ Wcat
    conf = _sigmoid(Q[:, 0:3])
    diff = _sigmoid(Q[:, 3])
    cap = np.asarray(inp["capital_shares"], np.float32)
    bids = conf * cap[None, :] * diff[:, None]
    bids = bids + Q[:, 4:7]
    return bids, conf


# ---------------------------------------------------------------- kernel


def kernel(**inputs):
    x = np.ascontiguousarray(np.asarray(inputs["x"], np.float32))
    assert x.shape == (B, T, C), x.shape
    ln1w = np.asarray(inputs["ln1_w"], np.float32)
    ln1b = np.asarray(inputs["ln1_b"], np.float32)
    ln2w = np.asarray(inputs["ln2_w"], np.float32)
    ln2b = np.asarray(inputs["ln2_b"], np.float32)
    Wr = np.asarray(inputs["Wr"], np.float32)
    Wv = np.asarray(inputs["Wv"], np.float32)
    Wo = np.asarray(inputs["Wo"], np.float32)
    Ws = np.asarray(inputs["Ws"], np.float32)
    K_rwkv = np.asarray(inputs["K_rwkv"], np.float32)
    V_rwkv = np.asarray(inputs["V_rwkv"], np.float32)
    W1 = np.asarray(inputs["W1"], np.float32)
    W2 = np.asarray(inputs["W2"], np.float32)
    W3 = np.asarray(inputs["W3"], np.float32)

    trace = _trace_enabled()
    if trace:
        _install_trace_shims()
        LAST_EXEC_NS.clear()

    with_bias = bool(np.any(ln1b))
    nc1, nc2 = _get_programs(with_bias)
    xf = x.reshape(N, C)

    # ---- launch 1
    def _wchunk(W):
        # [p, kc, c] with element W[kc*128+p, c]
        return np.ascontiguousarray(
            W.reshape(NK, 128, C).transpose(1, 0, 2)
        ).astype(BF16_NP)

    wrp = _wchunk(ln1w[:, None] * Wr)
    wvp = _wchunk(ln1w[:, None] * Wv)
    wop = _wchunk(Wo)
    in1 = []
    for c in range(NCORES):
        d = {
            "x": xf[c * TLOC:(c + 1) * TLOC],
            "wr": wrp, "wv": wvp, "wo": wop,
        }
        if with_bias:
            d["brow"] = np.ascontiguousarray(
                np.stack([ln1b @ Wr, ln1b @ Wv]).astype(np.float32)
            ).astype(BF16_NP)
        in1.append(d)
    res1 = run_bass_kernel_spmd(nc1, in1, list(range(NCORES)), trace=trace)
    if trace:
        LAST_EXEC_NS.append(res1.exec_time_ns)
    x2 = np.concatenate([res1.results[c]["x2"] for c in range(NCORES)], axis=0)

    # ---- host: LN2, z1, routing
    h = _ln_np(x2, ln2w, ln2b)
    z1 = _ln_pre_np(xf)
    bids, conf = _routing_from_h(h, inputs)
    order = np.argsort(bids, axis=1)
    winners = order[:, 2].astype(np.int64)
    gap = np.take_along_axis(bids, order[:, 2:3], 1)[:, 0] - np.take_along_axis(
        bids, order[:, 1:2], 1
    )[:, 0]
    margin_idx = np.nonzero(gap < MARGIN)[0]

    # exact recompute of borderline tokens (fp32, reference order)
    exact = {}
    if margin_idx.size:
        xr = xf[margin_idx]
        xln = _ln_np(xr, ln1w, ln1b)
        att = (_sigmoid(xln @ Wr) * (xln @ Wv)) @ Wo
        x2e = xr + att
        he = _ln_np(x2e, ln2w, ln2b)
        ste = xln @ Ws
        bide, confe = _routing_from_h(he, inputs)
        we = np.argmax(bide, axis=1)
        wce = np.take_along_axis(confe, we[:, None], 1)[:, 0]
        sce = wce / (wce + np.float32(1e-6))
        oute = _expert_out_host(he, ste, we, K_rwkv, V_rwkv, W1, W2, W3)
        for j, t in enumerate(margin_idx):
            exact[int(t)] = x2e[j] + oute[j] * sce[j]

    win_conf = np.take_along_axis(conf, winners[:, None], 1)[:, 0]
    scale = win_conf / (win_conf + np.float32(1e-6))

    # ---- pack tokens for launch 2
    is_margin = np.zeros(N, bool)
    is_margin[margin_idx] = True
    host_extra = []  # (token, winner) computed on host

    # one rwkv expert per core; greedy: bigger expert first
    counts = [np.nonzero((winners == e) & ~is_margin)[0] for e in (0, 1)]
    core_r = [None] * NCORES   # per-core (idx_array, expert)
    free_cores = list(range(NCORES))
    for e in sorted((0, 1), key=lambda e: -counts[e].size):
        idx = counts[e]
        pos = 0
        while pos < idx.size and free_cores:
            cidx = free_cores.pop(0)
            take = min(CAP_R, idx.size - pos)
            core_r[cidx] = (idx[pos:pos + take], e)
            pos += take
        if pos < idx.size:
            host_extra.extend((int(t), e) for t in idx[pos:])

    idx_t = np.nonzero((winners == 2) & ~is_margin)[0]
    if idx_t.size > NCORES * CAP_T:
        host_extra.extend((int(t), 2) for t in idx_t[NCORES * CAP_T:])
        idx_t = idx_t[:NCORES * CAP_T]
    per = (idx_t.size + NCORES - 1) // NCORES if idx_t.size else 0
    core_t = [idx_t[c * per:(c + 1) * per] for c in range(NCORES)]

    hbf = h.astype(BF16_NP)
    z1bf = z1.astype(BF16_NP)

    def _wchunk_l2(W):
        # [kc, p, c] bf16 chunk-lhsT layout
        return np.ascontiguousarray(
            W.reshape(NK, 128, NK, 128).transpose(2, 1, 0, 3).reshape(NK, 128, C)
        ).astype(BF16_NP)

    k_bf = {
        e: np.ascontiguousarray(
            K_rwkv[e].reshape(NK, 128, NH, 128).transpose(2, 1, 0, 3).reshape(
                NH, 128, C
            )
        ).astype(BF16_NP)
        for e in (0, 1)
    }
    v_bf = {e: np.ascontiguousarray(V_rwkv[e]).astype(BF16_NP) for e in (0, 1)}
    w1c = _wchunk_l2(W1)
    w2c = _wchunk_l2((ln1w[:, None] * Ws) @ W2)
    w3b = np.ascontiguousarray(
        W3.reshape(NK, 128, C).transpose(1, 0, 2)
    ).astype(BF16_NP)
    bsrow = np.ascontiguousarray((ln1b @ Ws @ W2).astype(np.float32))

    def _pack_T(mat_rows, cap):
        # rows [cnt, C] -> [128, NK, cap] with (p, k, t) = rows[t, k*128+p]
        out = np.zeros((128, NK, cap), BF16_NP)
        cnt = mat_rows.shape[0]
        if cnt:
            out[:, :, :cnt] = mat_rows.T.reshape(NK, 128, cnt).transpose(1, 0, 2)
        return out

    empty = np.empty(0, np.int64)
    in2 = []
    for c in range(NCORES):
        idx_r, er = core_r[c] if core_r[c] is not None else (empty, 0)
        ti = core_t[c]
        in2.append(
            {
                "ht": _pack_T(hbf[idx_r], CAP_R),
                "k2": k_bf[er], "v2": v_bf[er],
                "w1": w1c, "w2": w2c, "w3": w3b,
                "bsr": bsrow,
                "htt": _pack_T(hbf[ti], CAP_T),
                "z1tp": _pack_T(z1bf[ti], CAP_T),
            }
        )
    res2 = run_bass_kernel_spmd(nc2, in2, list(range(NCORES)), trace=trace)
    if trace:
        LAST_EXEC_NS.append(res2.exec_time_ns)

    # ---- combine
    y = x2.copy()
    for c in range(NCORES):
        idx_r, _ = core_r[c] if core_r[c] is not None else (empty, 0)
        if idx_r.size:
            y[idx_r] += (
                res2.results[c]["outr"][:idx_r.size].astype(np.float32)
                * scale[idx_r, None]
            )
        ti = core_t[c]
        if ti.size:
            y[ti] += (
                res2.results[c]["outt"][:ti.size].astype(np.float32)
                * scale[ti, None]
            )

    if host_extra:
        toks = np.array([t for t, _ in host_extra], np.int64)
        wv_ = winners[toks]
        xln_rows = z1[toks] * ln1w + ln1b
        st_rows = xln_rows @ Ws
        out_h = _expert_out_host(
            h[toks], st_rows, wv_, K_rwkv, V_rwkv, W1, W2, W3
        )
        y[toks] += out_h * scale[toks, None]

    for t, row in exact.items():
        y[t] = row

    return np.ascontiguousarray(y.reshape(B, T, C).astype(np.float32))


# revision 14
# speedup vs baseline: 1.2584x; 1.1054x over previous
"""Trainium2 Bass kernel for nn_CaMoE_Block (MoE routing block).

Strategy (8 NeuronCores, 2 launches, host routing between):
  Launch 1 -- data-parallel over tokens (8192/8 = 1024 per core), all bf16:
    LN1 (pre-affine, ln1_w folded into weights) -> DMA-XBAR transpose ->
    gated attention matmuls in [token, C] output layout (z stationary,
    weights moving; 512-wide moving rows) -> residual x2 = x + att.
    Exports ONLY x2 (f32); host recomputes LN2/z1 from x/x2.
  Host -- routing: h = LN2(x2)*w+b, bids, winners; borderline tokens
    (top-2 gap < MARGIN) recomputed exactly in fp32 reference order.
    Packing: one rwkv expert per core (5 cores e0 / 3 cores e1 for the
    reference distribution); overflow tokens computed exactly on host.
  Launch 2 -- expert-parallel: each core gets ONE rwkv expert's K/V plus
    the shared transformer-expert weights. R1 (squared-relu K matmul,
    640-token slot) -> transformer expert (416-token slot) -> R2 (V
    matmul, V resident in SBUF). Outputs bf16.
  Host -- scale by straight-through confidence, scatter-add residual.
"""

import os
import sys

sys.path.insert(0, "/opt/trn_rl_repo")

from contextlib import ExitStack

import ml_dtypes
import numpy as np

import concourse.bacc as bacc
import concourse.tile as tile
from concourse import mybir
from concourse.bass_utils import run_bass_kernel_spmd
from concourse.masks import make_identity

F32 = mybir.dt.float32
BF16 = mybir.dt.bfloat16
BF16_NP = ml_dtypes.bfloat16
AF = mybir.ActivationFunctionType
ALU = mybir.AluOpType

B, T, C = 4, 2048, 1024
N = B * T                      # 8192 tokens
NCORES = 8
TLOC = N // NCORES             # 1024 tokens per core
H = 4 * C                      # 4096
NK = C // 128                  # 8
NT = TLOC // 128               # 8
NH = H // 128                  # 32
CAP_R = 640                    # rwkv tokens per core in launch 2
CAP_T = 416                    # transformer tokens per core in launch 2
MARGIN = 3e-3                  # top-2 bid gap below which host recomputes
LN_EPS = 1e-5

# populated when BASS_MOE_TRACE=1: [launch1_ns, launch2_ns]
LAST_EXEC_NS = []

_CACHE = {}


def _trace_enabled():
    return bool(int(os.environ.get("BASS_MOE_TRACE", "0")))


def _install_trace_shims():
    """This image lacks antenv.axon_hooks; synthesize it so trace=True works."""
    import types

    import antenv
    import concourse.bass_utils as bass_utils

    if "antenv.axon_hooks" not in sys.modules:
        from trn_agent_boot.trn_boot import _ntff_profile_via_ctypes

        mod = types.ModuleType("antenv.axon_hooks")
        hook = _ntff_profile_via_ctypes("/opt/axon/libaxon_pjrt.so")
        mod.get_axon_ntff_profile_hook = lambda: hook
        mod.set_axon_ntff_profile_hook = lambda h: None
        sys.modules["antenv.axon_hooks"] = mod
        antenv.axon_hooks = mod
    bass_utils.upload_artifacts = lambda tmpdir: "local://" + tmpdir


# ---------------------------------------------------------------- launch 1


def _build_launch1(with_bias):
    nc = bacc.Bacc()
    x = nc.declare_dram_parameter("x", [TLOC, C], F32, isOutput=False)
    # weights: [p, kc, c] with element W'[kc*128+p, c]  (W' = ln1w-folded)
    wr = nc.declare_dram_parameter("wr", [128, NK, C], BF16, isOutput=False)
    wv = nc.declare_dram_parameter("wv", [128, NK, C], BF16, isOutput=False)
    wo = nc.declare_dram_parameter("wo", [128, NK, C], BF16, isOutput=False)
    if with_bias:
        # rows b@Wr', b@Wv' as [1, C] each
        brow = nc.declare_dram_parameter("brow", [2, C], BF16, isOutput=False)
    x2 = nc.declare_dram_parameter("x2", [TLOC, C], F32, isOutput=True)

    with tile.TileContext(nc) as tc, ExitStack() as ctx:
        big = ctx.enter_context(tc.tile_pool(name="big", bufs=1))
        io = ctx.enter_context(tc.tile_pool(name="io", bufs=3))
        stat = ctx.enter_context(tc.tile_pool(name="stat", bufs=4))
        prv = ctx.enter_context(tc.tile_pool(name="prv", bufs=4, space="PSUM"))
        patt = ctx.enter_context(tc.tile_pool(name="patt", bufs=2, space="PSUM"))
        ptr = ctx.enter_context(tc.tile_pool(name="ptr", bufs=2, space="PSUM"))

        eps_t = big.tile([128, 1], F32)
        nc.vector.memset(eps_t, LN_EPS)
        identf = big.tile([128, 128], F32)
        make_identity(nc, identf)
        identb = big.tile([128, 128], BF16)
        nc.vector.tensor_copy(out=identb, in_=identf)

        # DMA service order == issue order (one logical FIFO over 16 rings):
        # issue critical-path transfers first, bulk later.
        xfull = big.tile([128, NT, C], F32)
        wrS = big.tile([128, NK, C], BF16)
        wvS = big.tile([128, NK, C], BF16)
        woS = big.tile([128, NK, C], BF16)
        xr_ap = x.rearrange("(i p) c -> p i c", p=128)
        for j in range(4):
            cs = slice(j * 256, (j + 1) * 256)
            nc.sync.dma_start(out=xfull[:, 0, cs], in_=xr_ap[:, 0, cs])
        for half in range(2):
            hs = slice(half * 512, (half + 1) * 512)
            nc.sync.dma_start(out=wrS[:, 0, hs], in_=wr[:, 0, hs])
            nc.sync.dma_start(out=wvS[:, 0, hs], in_=wv[:, 0, hs])
        for j in range(2):
            cs = slice(j * 512, (j + 1) * 512)
            nc.sync.dma_start(out=xfull[:, 1, cs], in_=xr_ap[:, 1, cs])
        if with_bias:
            brS = big.tile([128, 2, C], BF16)
            nc.sync.dma_start(
                out=brS[0:1], in_=brow.rearrange("b c -> 1 b c")
            )
            onesS = big.tile([1, 128], BF16)
            nc.vector.memset(onesS, 1.0)

        def layer_norm_pre(xt, tag):
            """-> z = (x - mean) * rstd as a fresh [128, C] bf16 tile."""
            stats = stat.tile([128, 2, 6], F32, tag=f"st_{tag}")
            nc.vector.bn_stats(out=stats[:, 0, :], in_=xt[:, 0:512])
            nc.vector.bn_stats(out=stats[:, 1, :], in_=xt[:, 512:1024])
            mv = stat.tile([128, 2], F32, tag=f"mv_{tag}")
            nc.vector.bn_aggr(out=mv, in_=stats)
            rstd = stat.tile([128, 1], F32, tag=f"rs_{tag}")
            nc.scalar.activation(out=rstd, in_=mv[:, 1:2], func=AF.Sqrt, bias=eps_t)
            nc.vector.reciprocal(out=rstd, in_=rstd)
            zt = io.tile([128, C], BF16, tag=f"z_{tag}")
            nc.vector.tensor_scalar(
                out=zt, in0=xt, scalar1=mv[:, 0:1], scalar2=rstd,
                op0=ALU.subtract, op1=ALU.mult,
            )
            return zt

        def transpose8(src_t, tag):
            """[128, C] bf16 -> [128, NK, 128] via PE transposes."""
            dst = io.tile([128, NK, 128], BF16, tag=tag)
            for k in range(NK):
                pt = ptr.tile([128, 128], BF16, tag="ptr")
                nc.tensor.transpose(pt, src_t[:, k * 128:(k + 1) * 128], identb)
                nc.scalar.activation(out=dst[:, k, :], in_=pt, func=AF.Copy)
            return dst

        # software pipeline over token tiles:
        #   stage A(i): LN1 -> zT (PE transpose) -> r/v matmuls
        #   stage B(i): evict r,v -> a=r*v -> aT -> att matmul -> x2
        pr = {}

        def stage_a(i):
            zt = layer_norm_pre(xfull[:, i, :], "ln1")
            zT = transpose8(zt, "zT")
            if i == 0:
                # bulk: rest of wr/wv, x tile 2, wo
                for k in range(1, NK):
                    for half in range(2):
                        hs = slice(half * 512, (half + 1) * 512)
                        nc.sync.dma_start(out=wrS[:, k, hs], in_=wr[:, k, hs])
                        nc.sync.dma_start(out=wvS[:, k, hs], in_=wv[:, k, hs])
                for j in range(2):
                    cs = slice(j * 512, (j + 1) * 512)
                    nc.sync.dma_start(out=xfull[:, 2, cs], in_=xr_ap[:, 2, cs])
                for k in range(NK):
                    nc.sync.dma_start(out=woS[:, k, :], in_=wo[:, k, :])
            elif i + 2 < NT:
                for j in range(2):
                    cs = slice(j * 512, (j + 1) * 512)
                    nc.sync.dma_start(
                        out=xfull[:, i + 2, cs], in_=xr_ap[:, i + 2, cs]
                    )
            ps = [
                prv.tile([128, 512], F32, tag="prv", name=f"prv_{i}_{j}")
                for j in range(4)
            ]
            for k in range(NK):
                st = dict(start=(k == 0), stop=(k == NK - 1 and not with_bias))
                nc.tensor.matmul(
                    ps[0], zT[:, k, :], wrS[:, k, 0:512],
                    skip_group_check=True, **st,
                )
                nc.tensor.matmul(
                    ps[1], zT[:, k, :], wrS[:, k, 512:1024],
                    skip_group_check=True, **st,
                )
                nc.tensor.matmul(
                    ps[2], zT[:, k, :], wvS[:, k, 0:512],
                    skip_group_check=True, **st,
                )
                nc.tensor.matmul(
                    ps[3], zT[:, k, :], wvS[:, k, 512:1024],
                    skip_group_check=True, **st,
                )
            if with_bias:
                for j in range(4):
                    nc.tensor.matmul(
                        ps[j], onesS,
                        brS[0:1, j // 2, (j % 2) * 512:(j % 2 + 1) * 512],
                        start=False, stop=True, skip_group_check=True,
                    )
            pr[i] = ps

        def stage_b(i):
            ps = pr.pop(i)
            rv = io.tile([128, 2, C], BF16, tag="rv")
            nc.scalar.activation(out=rv[:, 0, 0:512], in_=ps[0], func=AF.Sigmoid)
            nc.scalar.activation(out=rv[:, 0, 512:1024], in_=ps[1], func=AF.Sigmoid)
            nc.vector.tensor_copy(out=rv[:, 1, 0:512], in_=ps[2])
            nc.vector.tensor_copy(out=rv[:, 1, 512:1024], in_=ps[3])
            at = io.tile([128, C], BF16, tag="amul")
            nc.vector.tensor_mul(out=at, in0=rv[:, 0, :], in1=rv[:, 1, :])
            aT = transpose8(at, "aT")
            pa = [
                patt.tile([128, 512], F32, tag="patt", name=f"patt_{i}_{j}")
                for j in range(2)
            ]
            for k in range(NK):
                st = dict(start=(k == 0), stop=(k == NK - 1))
                nc.tensor.matmul(
                    pa[0], aT[:, k, :], woS[:, k, 0:512],
                    skip_group_check=True, **st,
                )
                nc.tensor.matmul(
                    pa[1], aT[:, k, :], woS[:, k, 512:1024],
                    skip_group_check=True, **st,
                )
            x2t = io.tile([128, C], F32, tag="x2t")
            nc.vector.tensor_add(out=x2t[:, 0:512], in0=xfull[:, i, 0:512], in1=pa[0])
            nc.sync.dma_start(
                out=x2[i * 128:(i + 1) * 128, 0:512], in_=x2t[:, 0:512]
            )
            nc.vector.tensor_add(
                out=x2t[:, 512:1024], in0=xfull[:, i, 512:1024], in1=pa[1]
            )
            nc.sync.dma_start(
                out=x2[i * 128:(i + 1) * 128, 512:1024], in_=x2t[:, 512:1024]
            )

        stage_a(0)
        for i in range(1, NT):
            stage_a(i)
            stage_b(i - 1)
        stage_b(NT - 1)

    nc.finalize()
    return nc


# ---------------------------------------------------------------- launch 2


def _build_launch2():
    nc = bacc.Bacc()
    # host-prepared layouts:
    #   ht/htt/z1tp: [128, NK, CAP]  (p, k, t) = M[k*128+p, t]
    #   k2: [NH, 128, C]  (hc, p, k*128+c) = K[k*128+p, hc*128+c]
    #   w1/w2: [NK, 128, C] chunk-lhsT;  w3: [128, NK, C]
    #   v2: [H, C]
    ht = nc.declare_dram_parameter("ht", [128, NK, CAP_R], BF16, isOutput=False)
    k2 = nc.declare_dram_parameter("k2", [NH, 128, C], BF16, isOutput=False)
    v2 = nc.declare_dram_parameter("v2", [H, C], BF16, isOutput=False)
    w1 = nc.declare_dram_parameter("w1", [NK, 128, C], BF16, isOutput=False)
    w2 = nc.declare_dram_parameter("w2", [NK, 128, C], BF16, isOutput=False)
    w3 = nc.declare_dram_parameter("w3", [128, NK, C], BF16, isOutput=False)
    htt = nc.declare_dram_parameter("htt", [128, NK, CAP_T], BF16, isOutput=False)
    z1tp = nc.declare_dram_parameter("z1tp", [128, NK, CAP_T], BF16, isOutput=False)
    bsr = nc.declare_dram_parameter("bsr", [C], F32, isOutput=False)
    outr = nc.declare_dram_parameter("outr", [CAP_R, C], BF16, isOutput=True)
    outt = nc.declare_dram_parameter("outt", [CAP_T, C], BF16, isOutput=True)

    CR0 = 512                  # R1 psum split: 512 + 128
    CR1 = CAP_R - CR0
    PRE = 4                    # kt DMA lookahead in R1

    with tile.TileContext(nc) as tc, ExitStack() as ctx:
        big = ctx.enter_context(tc.tile_pool(name="big", bufs=1))
        stream = ctx.enter_context(tc.tile_pool(name="stream", bufs=6))
        wst = ctx.enter_context(tc.tile_pool(name="wst", bufs=4))
        ev = ctx.enter_context(tc.tile_pool(name="ev", bufs=3))
        ps = ctx.enter_context(tc.tile_pool(name="ps", bufs=6, space="PSUM"))

        hT = big.tile([128, NK, CAP_R], BF16)
        hr = big.tile([128, NH, CAP_R], BF16)
        v2sb = big.tile([128, NH, C], BF16)
        w3sb = big.tile([128, NK, C], BF16)
        hTt = big.tile([128, NK, CAP_T], BF16)
        z1T = big.tile([128, NK, CAP_T], BF16)
        gT = big.tile([128, NK, CAP_T], BF16)
        bst = big.tile([128, NK], F32)

        # critical first: hT, then the first PRE kt tiles
        for k in range(NK):
            nc.sync.dma_start(out=hT[:, k, :], in_=ht[:, k, :])
        kts = {}

        def issue_kt(hc):
            if hc >= NH:
                return
            kt = stream.tile([128, NK, 128], BF16, tag="kt", name=f"kt_{hc}")
            k2r = k2[hc].rearrange("p (k c) -> p k c", c=128)
            nc.sync.dma_start(out=kt[:, 0:4, :], in_=k2r[:, 0:4, :])
            nc.sync.dma_start(out=kt[:, 4:8, :], in_=k2r[:, 4:8, :])
            kts[hc] = kt

        for hc in range(PRE):
            issue_kt(hc)
        nc.sync.dma_start(out=bst, in_=bsr.rearrange("(m p) -> p m", p=128))

        # bulk prefetch thunks, drained 1-2 per R1 iteration
        v2r = v2.rearrange("(hc p) c -> p hc c", p=128)
        bulk = []
        for g in range(NH):
            bulk.append((v2sb[:, g, :], v2r[:, g, :]))
        for g in range(NK):
            bulk.append((w3sb[:, g, :], w3[:, g, :]))
        bulk.append((hTt[:, 0:4, :], htt[:, 0:4, :]))
        bulk.append((hTt[:, 4:8, :], htt[:, 4:8, :]))
        bulk.append((z1T[:, 0:4, :], z1tp[:, 0:4, :]))
        bulk.append((z1T[:, 4:8, :], z1tp[:, 4:8, :]))
        bulk.reverse()  # pop() from the front order

        # transformer-expert weight stream with lookahead
        wts = {}

        def issue_w(cc):
            if cc >= NK:
                return
            t1 = wst.tile([128, NK, 128], BF16, tag="w1t", name=f"w1t_{cc}")
            w1r = w1[cc].rearrange("p (k c) -> p k c", c=128)
            nc.sync.dma_start(out=t1[:, 0:4, :], in_=w1r[:, 0:4, :])
            nc.sync.dma_start(out=t1[:, 4:8, :], in_=w1r[:, 4:8, :])
            t2 = wst.tile([128, NK, 128], BF16, tag="w2t", name=f"w2t_{cc}")
            w2r = w2[cc].rearrange("p (k c) -> p k c", c=128)
            nc.sync.dma_start(out=t2[:, 0:4, :], in_=w2r[:, 0:4, :])
            nc.sync.dma_start(out=t2[:, 4:8, :], in_=w2r[:, 4:8, :])
            wts[cc] = (t1, t2)

        # ---- R1: hr = relu(h @ K)^2, output [hc, token] layout
        for hc in range(NH):
            issue_kt(hc + PRE)
            if bulk:
                nc.sync.dma_start(*bulk.pop())
            if hc % 2 == 0 and bulk:
                nc.sync.dma_start(*bulk.pop())
            if hc == NH - 2:
                issue_w(0)
            if hc == NH - 1:
                issue_w(1)
            kt = kts.pop(hc)
            pa = ps.tile([128, 512], F32, tag="ps", name=f"r1a_{hc}")
            pb = ps.tile([128, 512], F32, tag="ps", name=f"r1b_{hc}")
            for k in range(NK):
                st = dict(start=(k == 0), stop=(k == NK - 1))
                nc.tensor.matmul(
                    pa, kt[:, k, :], hT[:, k, 0:CR0], skip_group_check=True, **st
                )
                nc.tensor.matmul(
                    pb[:, 0:CR1], kt[:, k, :], hT[:, k, CR0:CAP_R],
                    skip_group_check=True, **st,
                )
            rel = ev.tile([128, CAP_R], F32, tag="rel")
            nc.scalar.activation(out=rel[:, 0:CR0], in_=pa, func=AF.Relu)
            nc.scalar.activation(
                out=rel[:, CR0:CAP_R], in_=pb[:, 0:CR1], func=AF.Relu
            )
            nc.vector.tensor_mul(
                out=hr[:, hc, 0:CR0], in0=rel[:, 0:CR0], in1=rel[:, 0:CR0]
            )
            nc.vector.tensor_mul(
                out=hr[:, hc, CR0:CAP_R], in0=rel[:, CR0:CAP_R],
                in1=rel[:, CR0:CAP_R],
            )

        while bulk:
            nc.sync.dma_start(*bulk.pop())

        # ---- T: transformer expert (state-gated)
        for cc in range(NK):
            issue_w(cc + 2)
            w1t, w2t = wts.pop(cc)
            psa = ps.tile([128, 512], F32, tag="pst", bufs=2, name=f"ta_{cc}")
            for k in range(NK):
                nc.tensor.matmul(
                    psa[:, :CAP_T], w1t[:, k, :], hTt[:, k, :],
                    start=(k == 0), stop=(k == NK - 1), skip_group_check=True,
                )
            at = ev.tile([128, 512], F32, tag="at")
            nc.vector.tensor_copy(out=at[:, :CAP_T], in_=psa[:, :CAP_T])

            psb = ps.tile([128, 512], F32, tag="pst", bufs=2, name=f"tg_{cc}")
            for k in range(NK):
                nc.tensor.matmul(
                    psb[:, :CAP_T], w2t[:, k, :], z1T[:, k, :],
                    start=(k == 0), stop=(k == NK - 1), skip_group_check=True,
                )
            sg = ev.tile([128, 512], F32, tag="sg")
            nc.scalar.activation(
                out=sg[:, :CAP_T], in_=psb[:, :CAP_T], func=AF.Sigmoid,
                bias=bst[:, cc:cc + 1],
            )
            nc.vector.tensor_mul(
                out=gT[:, cc, :], in0=at[:, :CAP_T], in1=sg[:, :CAP_T]
            )

        tspans = [(0, 128), (128, 128), (256, 128), (384, CAP_T - 384)]
        for t0, tsz in tspans:
            for cn in range(2):
                pst = ps.tile([128, 512], F32, tag="pst", bufs=2, name=f"t3_{t0}_{cn}")
                for k in range(NK):
                    nc.tensor.matmul(
                        pst[:tsz], gT[:, k, t0:t0 + tsz],
                        w3sb[:, k, cn * 512:(cn + 1) * 512],
                        start=(k == 0), stop=(k == NK - 1), skip_group_check=True,
                    )
                oev = ev.tile([128, 512], BF16, tag="oev", name=f"t3ev_{t0}_{cn}")
                nc.vector.tensor_copy(out=oev[:tsz], in_=pst[:tsz])
                nc.sync.dma_start(
                    out=outt[t0:t0 + tsz, cn * 512:(cn + 1) * 512], in_=oev[:tsz]
                )

        # ---- R2: out_r = hr^T @ V, V resident; two cn passes of 5 banks
        for cn in range(2):
            psts = [
                ps.tile([128, 512], F32, tag="ps", name=f"r2_{cn}_{tt}")
                for tt in range(5)
            ]
            for hc in range(NH):
                for tt in range(5):
                    nc.tensor.matmul(
                        psts[tt], hr[:, hc, tt * 128:(tt + 1) * 128],
                        v2sb[:, hc, cn * 512:(cn + 1) * 512],
                        start=(hc == 0), stop=(hc == NH - 1),
                        skip_group_check=True,
                    )
            for tt in range(5):
                oev = ev.tile([128, 512], BF16, tag="oev", name=f"r2ev_{cn}_{tt}")
                nc.vector.tensor_copy(out=oev, in_=psts[tt])
                nc.sync.dma_start(
                    out=outr[tt * 128:(tt + 1) * 128, cn * 512:(cn + 1) * 512],
                    in_=oev,
                )

    nc.finalize()
    return nc


def _get_programs(with_bias):
    key1 = f"nc1_{with_bias}"
    if key1 not in _CACHE:
        _CACHE[key1] = _build_launch1(with_bias)
    if "nc2" not in _CACHE:
        _CACHE["nc2"] = _build_launch2()
    return _CACHE[key1], _CACHE["nc2"]


# ---------------------------------------------------------------- host math


def _sigmoid(x):
    return 1.0 / (1.0 + np.exp(-x.astype(np.float32), dtype=np.float32))


def _ln_np(x, w, b):
    x = x.astype(np.float32)
    m = x.mean(axis=-1, keepdims=True, dtype=np.float32)
    v = x.var(axis=-1, keepdims=True, dtype=np.float32)
    return ((x - m) / np.sqrt(v + np.float32(LN_EPS)) * w + b).astype(np.float32)


def _ln_pre_np(x):
    x = x.astype(np.float32)
    m = x.mean(axis=-1, keepdims=True, dtype=np.float32)
    v = x.var(axis=-1, keepdims=True, dtype=np.float32)
    return ((x - m) / np.sqrt(v + np.float32(LN_EPS))).astype(np.float32)


def _expert_out_host(hrows, strows, wvec, K_rwkv, V_rwkv, W1, W2, W3):
    """Exact fp32 expert outputs for a small token batch (reference order)."""
    out = np.zeros((hrows.shape[0], C), np.float32)
    for e in (0, 1):
        m = wvec == e
        if m.any():
            z = hrows[m] @ K_rwkv[e]
            hr = np.square(np.maximum(z, 0.0))
            out[m] = hr @ V_rwkv[e]
    m = wvec == 2
    if m.any():
        out[m] = ((hrows[m] @ W1) * _sigmoid(strows[m] @ W2)) @ W3
    return out


def _routing_from_h(h, inp):
    """bids (N,3) in reference op order."""
    Wcat = np.concatenate(
        [
            np.asarray(inp["conf_rwkv"], np.float32).T,
            np.asarray(inp["conf_trans"], np.float32)[:, None],
            np.asarray(inp["w_diff"], np.float32)[:, None],
            np.asarray(inp["W_aff"], np.float32),
        ],
        axis=1,
    )
    Q = h @ Wcat
    conf = _sigmoid(Q[:, 0:3])
    diff = _sigmoid(Q[:, 3])
    cap = np.asarray(inp["capital_shares"], np.float32)
    bids = conf * cap[None, :] * diff[:, None]
    bids = bids + Q[:, 4:7]
    return bids, conf


# ---------------------------------------------------------------- kernel


def kernel(**inputs):
    x = np.ascontiguousarray(np.asarray(inputs["x"], np.float32))
    assert x.shape == (B, T, C), x.shape
    ln1w = np.asarray(inputs["ln1_w"], np.float32)
    ln1b = np.asarray(inputs["ln1_b"], np.float32)
    ln2w = np.asarray(inputs["ln2_w"], np.float32)
    ln2b = np.asarray(inputs["ln2_b"], np.float32)
    Wr = np.asarray(inputs["Wr"], np.float32)
    Wv = np.asarray(inputs["Wv"], np.float32)
    Wo = np.asarray(inputs["Wo"], np.float32)
    Ws = np.asarray(inputs["Ws"], np.float32)
    K_rwkv = np.asarray(inputs["K_rwkv"], np.float32)
    V_rwkv = np.asarray(inputs["V_rwkv"], np.float32)
    W1 = np.asarray(inputs["W1"], np.float32)
    W2 = np.asarray(inputs["W2"], np.float32)
    W3 = np.asarray(inputs["W3"], np.float32)

    trace = _trace_enabled()
    if trace:
        _install_trace_shims()
        LAST_EXEC_NS.clear()

    with_bias = bool(np.any(ln1b))
    nc1, nc2 = _get_programs(with_bias)
    xf = x.reshape(N, C)

    # ---- launch 1
    def _wchunk(W):
        # [p, kc, c] with element W[kc*128+p, c]
        return np.ascontiguousarray(
            W.reshape(NK, 128, C).transpose(1, 0, 2)
        ).astype(BF16_NP)

    wrp = _wchunk(ln1w[:, None] * Wr)
    wvp = _wchunk(ln1w[:, None] * Wv)
    wop = _wchunk(Wo)
    in1 = []
    for c in range(NCORES):
        d = {
            "x": xf[c * TLOC:(c + 1) * TLOC],
            "wr": wrp, "wv": wvp, "wo": wop,
        }
        if with_bias:
            d["brow"] = np.ascontiguousarray(
                np.stack([ln1b @ Wr, ln1b @ Wv]).astype(np.float32)
            ).astype(BF16_NP)
        in1.append(d)
    res1 = run_bass_kernel_spmd(nc1, in1, list(range(NCORES)), trace=trace)
    if trace:
        LAST_EXEC_NS.append(res1.exec_time_ns)
    x2 = np.concatenate([res1.results[c]["x2"] for c in range(NCORES)], axis=0)

    # ---- host: LN2, z1, routing
    h = _ln_np(x2, ln2w, ln2b)
    z1 = _ln_pre_np(xf)
    bids, conf = _routing_from_h(h, inputs)
    order = np.argsort(bids, axis=1)
    winners = order[:, 2].astype(np.int64)
    gap = np.take_along_axis(bids, order[:, 2:3], 1)[:, 0] - np.take_along_axis(
        bids, order[:, 1:2], 1
    )[:, 0]
    margin_idx = np.nonzero(gap < MARGIN)[0]

    # exact recompute of borderline tokens (fp32, reference order)
    exact = {}
    if margin_idx.size:
        xr = xf[margin_idx]
        xln = _ln_np(xr, ln1w, ln1b)
        att = (_sigmoid(xln @ Wr) * (xln @ Wv)) @ Wo
        x2e = xr + att
        he = _ln_np(x2e, ln2w, ln2b)
        ste = xln @ Ws
        bide, confe = _routing_from_h(he, inputs)
        we = np.argmax(bide, axis=1)
        wce = np.take_along_axis(confe, we[:, None], 1)[:, 0]
        sce = wce / (wce + np.float32(1e-6))
        oute = _expert_out_host(he, ste, we, K_rwkv, V_rwkv, W1, W2, W3)
        for j, t in enumerate(margin_idx):
            exact[int(t)] = x2e[j] + oute[j] * sce[j]

    win_conf = np.take_along_axis(conf, winners[:, None], 1)[:, 0]
    scale = win_conf / (win_conf + np.float32(1e-6))

    # ---- pack tokens for launch 2
    is_margin = np.zeros(N, bool)
    is_margin[margin_idx] = True
    host_extra = []  # (token, winner) computed on host

    # one rwkv expert per core; greedy: bigger expert first
    counts = [np.nonzero((winners == e) & ~is_margin)[0] for e in (0, 1)]
    core_r = [None] * NCORES   # per-core (idx_array, expert)
    free_cores = list(range(NCORES))
    for e in sorted((0, 1), key=lambda e: -counts[e].size):
        idx = counts[e]
        pos = 0
        while pos < idx.size and free_cores:
            cidx = free_cores.pop(0)
            take = min(CAP_R, idx.size - pos)
            core_r[cidx] = (idx[pos:pos + take], e)
            pos += take
        if pos < idx.size:
            host_extra.extend((int(t), e) for t in idx[pos:])

    idx_t = np.nonzero((winners == 2) & ~is_margin)[0]
    if idx_t.size > NCORES * CAP_T:
        host_extra.extend((int(t), 2) for t in idx_t[NCORES * CAP_T:])
        idx_t = idx_t[:NCORES * CAP_T]
    per = (idx_t.size + NCORES - 1) // NCORES if idx_t.size else 0
    core_t = [idx_t[c * per:(c + 1) * per] for c in range(NCORES)]

    hbf = h.astype(BF16_NP)
    z1bf = z1.astype(BF16_NP)

    def _wchunk_l2(W):
        # [kc, p, c] bf16 chunk-lhsT layout
        return np.ascontiguousarray(
            W.reshape(NK, 128, NK, 128).transpose(2, 1, 0, 3).reshape(NK, 128, C)
        ).astype(BF16_NP)

    k_bf = {
        e: np.ascontiguousarray(
            K_rwkv[e].reshape(NK, 128, NH, 128).transpose(2, 1, 0, 3).reshape(
                NH, 128, C
            )
        ).astype(BF16_NP)
        for e in (0, 1)
    }
    v_bf = {e: np.ascontiguousarray(V_rwkv[e]).astype(BF16_NP) for e in (0, 1)}
    w1c = _wchunk_l2(W1)
    w2c = _wchunk_l2((ln1w[:, None] * Ws) @ W2)
    w3b = np.ascontiguousarray(
        W3.reshape(NK, 128, C).transpose(1, 0, 2)
    ).astype(BF16_NP)
    bsrow = np.ascontiguousarray((ln1b @ Ws @ W2).astype(np.float32))

    def _pack_T(mat_rows, cap):
        # rows [cnt, C] -> [128, NK, cap] with (p, k, t) = rows[t, k*128+p]
        out = np.zeros((128, NK, cap), BF16_NP)
        cnt = mat_rows.shape[0]
        if cnt:
            out[:, :, :cnt] = mat_rows.T.reshape(NK, 128, cnt).transpose(1, 0, 2)
        return out

    empty = np.empty(0, np.int64)
    in2 = []
    for c in range(NCORES):
        idx_r, er = core_r[c] if core_r[c] is not None else (empty, 0)
        ti = core_t[c]
        in2.append(
            {
                "ht": _pack_T(hbf[idx_r], CAP_R),
                "k2": k_bf[er], "v2": v_bf[er],
                "w1": w1c, "w2": w2c, "w3": w3b,
                "bsr": bsrow,
                "htt": _pack_T(hbf[ti], CAP_T),
                "z1tp": _pack_T(z1bf[ti], CAP_T),
            }
        )
    res2 = run_bass_kernel_spmd(nc2, in2, list(range(NCORES)), trace=trace)
    if trace:
        LAST_EXEC_NS.append(res2.exec_time_ns)

    # ---- combine
    y = x2.copy()
    for c in range(NCORES):
        idx_r, _ = core_r[c] if core_r[c] is not None else (empty, 0)
        if idx_r.size:
            y[idx_r] += (
                res2.results[c]["outr"][:idx_r.size].astype(np.float32)
                * scale[idx_r, None]
            )
        ti = core_t[c]
        if ti.size:
            y[ti] += (
                res2.results[c]["outt"][:ti.size].astype(np.float32)
                * scale[ti, None]
            )

    if host_extra:
        toks = np.array([t for t, _ in host_extra], np.int64)
        wv_ = winners[toks]
        xln_rows = z1[toks] * ln1w + ln1b
        st_rows = xln_rows @ Ws
        out_h = _expert_out_host(
            h[toks], st_rows, wv_, K_rwkv, V_rwkv, W1, W2, W3
        )
        y[toks] += out_h * scale[toks, None]

    for t, row in exact.items():
        y[t] = row

    return np.ascontiguousarray(y.reshape(B, T, C).astype(np.float32))
